# revision 37
# baseline (speedup 1.0000x reference)
"""2-layer GAT (PyG GATConv semantics) on 8 Trainium2 NeuronCores via Bass/Tile.

Sharding: B=2 graphs x 4 cores. Core (g,s) owns a 12500-node dst shard of
graph g, with dst nodes packed into 250 degree-balanced windows of 50 (LPT
assignment, host-side). x arrives PRE-PERMUTED per core (shard nodes first in
window order), so the compiled SPMD program is identical across cores and all
per-core structure lives in data (gather indices, window one-hot streams).

Per layer on each core:
  stage:  Z = x @ [W | W a_src | W a_dst] densely into a 512B-stride bf16
          DRAM array of rows [h | alpha_src | alpha_dst].
  edges:  per-edge rows fetched with dma_gather (bf16 rows; int16 indices).
          The index space is covered by TWO OVERLAPPING 32768-row halves
          (rows 0..32767 and 17232..49999); edges whose source falls in the
          overlap are assigned per-window so half0 carries exactly <=512 and
          half1 <=384 edges -> 7 gather tiles per window instead of 9.
          Gathers round-robin over 4 SWDGE queues: each queue's descriptor
          generation runs on a different Q7 cpu pair, so up to 4 gathers
          overlap (the single-queue desc-gen rate of ~8ns/row was the
          baseline bottleneck). The segment softmax folds into a one-hot
          matrix Mt[slot,node] = exp(leakyrelu(as+ad)) * (dst==node); one
          matmul per 128-edge tile accumulates [50, F+2] = (sum e*h | sum e)
          in PSUM; the epilogue divides by the denominator.
  L1->L2: shard results exchanged with piecewise AllGather (bf16), overlapped
          with the tail of the L1 edge phase.
"""
import sys
import numpy as np

sys.path.insert(0, "/opt/trn_rl_repo")

NEG_SLOPE = 0.2

FULL_CFG = dict(
    N=50000, B=2, D=128, HID=128, OUT=64,
    STRIPE=2500, WIN=50, CH_WIN=5, SPLIT=32768,
)


def _derive(cfg):
    c = dict(cfg)
    c["SHARD"] = c["N"] // 4
    c["NWIN"] = c["SHARD"] // c["WIN"]
    assert c["NWIN"] % c["CH_WIN"] == 0
    c["NCHUNK"] = c["NWIN"] // c["CH_WIN"]
    c["NPIECE"] = c["N"] // (4 * c["STRIPE"])
    assert c["STRIPE"] % c["WIN"] == 0
    assert c["NCHUNK"] % c["NPIECE"] == 0
    c["S2CH"] = 125 if c["STRIPE"] % 125 == 0 else c["WIN"]
    assert c["STRIPE"] % c["S2CH"] == 0
    c["WINR"] = c["CH_WIN"] * c["WIN"]
    assert c["STRIPE"] % c["WINR"] == 0
    c["OVL"] = c["N"] - c["SPLIT"]          # half1 base row
    assert c["OVL"] + c["SPLIT"] == c["N"] and c["OVL"] < c["SPLIT"]
    return c


def _stripe_ids(cfg, s):
    """Node ids owned by shard s (stripe pattern; set membership only)."""
    j = np.arange(cfg["SHARD"])
    return ((j // cfg["STRIPE"]) * 4 + s) * cfg["STRIPE"] + (j % cfg["STRIPE"])


def _ag_pos(cfg, s, j):
    """AllGather-output row of shard s's j-th node."""
    st = cfg["STRIPE"]
    return ((j // st) * 4 + s) * st + (j % st)


def _lpt_order(cfg, deg_shard):
    """Order shard nodes into NWIN windows of WIN, balancing total degree.

    deg_shard: [SHARD] degree of each shard node (position in stripe-id list).
    Returns ids_order: permutation of range(SHARD); node ids_order[j] goes to
    j-order slot j (window j//WIN).
    """
    import heapq
    WIN, NWIN = cfg["WIN"], cfg["NWIN"]
    order = np.argsort(-deg_shard, kind="stable")
    heap = [(0, w) for w in range(NWIN)]
    heapq.heapify(heap)
    fill = np.zeros(NWIN, dtype=np.int64)
    members = [[] for _ in range(NWIN)]
    for idx in order:
        while True:
            load, w = heapq.heappop(heap)
            if fill[w] < WIN:
                break
        members[w].append(idx)
        fill[w] += 1
        if fill[w] < WIN:
            heapq.heappush(heap, (load + int(deg_shard[idx]), w))
    out = np.empty(cfg["SHARD"], dtype=np.int64)
    k = 0
    for w in range(NWIN):
        assert len(members[w]) == WIN
        for idx in members[w]:
            out[k] = idx
            k += 1
    return out


def _wrap_idx(stream):
    n = len(stream)
    a = np.asarray(stream, dtype=np.int16).reshape(n // 16, 16).T
    return np.tile(a, (8, 1))


def _choose_T(cfg, stats):
    """stats: list of (f0, f1, tot) arrays over windows (all cores).
    Pick minimal (T0,T1) caps under the overlapping-halves split."""
    f0 = np.concatenate([s[0] for s in stats])
    f1 = np.concatenate([s[1] for s in stats])
    tot = np.concatenate([s[2] for s in stats])
    flex = tot - f0 - f1
    for T0, T1 in ((4, 3), (5, 3), (4, 4), (5, 4), (6, 4), (7, 4), (8, 5),
                   (10, 6), (14, 8)):
        a, b = T0 * 128, T1 * 128
        if (f0 <= a).all() and (f1 <= b).all() and (tot <= a + b).all() \
                and (f0 + flex >= tot - b).all():
            return T0, T1
    raise AssertionError("no feasible (T0,T1)")


def _assign_halves(cfg, T, w, pos, nwin):
    """Per-edge half assignment. Returns (half, eff_idx) arrays."""
    T0, T1 = T
    a, b = T0 * 128, T1 * 128
    OVL, SPLIT = cfg["OVL"], cfg["SPLIT"]
    forced0 = pos < OVL
    forced1 = pos >= SPLIT
    flex = ~forced0 & ~forced1
    half = np.where(forced1, 1, 0).astype(np.int64)
    # per window: put (d0 - f0) flex edges into half0
    worder = np.argsort(w, kind="stable")
    counts = np.bincount(w, minlength=nwin)
    starts = np.concatenate([[0], np.cumsum(counts)])
    for wi in range(nwin):
        widx = worder[starts[wi]:starts[wi + 1]]
        f0 = int(np.count_nonzero(forced0[widx]))
        fx_idx = widx[flex[widx]]
        tot = len(widx)
        d0 = min(a, f0 + len(fx_idx), tot)
        d0 = max(d0, tot - b, f0)
        take = d0 - f0
        assert 0 <= take <= len(fx_idx)
        half[fx_idx[take:]] = 1
    eff = pos - half * OVL
    assert (eff >= 0).all() and (eff < SPLIT).all()
    return half, eff


def _layer_streams(cfg, w, loc, half, eff, T0, T1):
    """Build gidx (wrapped int16) + dstloc streams for one layer."""
    WIN = cfg["WIN"]
    NWIN, CH_WIN, NCHUNK = cfg["NWIN"], cfg["CH_WIN"], cfg["NCHUNK"]
    TW = T0 + T1
    order = np.lexsort((half, w))
    ef, w2, loc2, hf = eff[order], w[order], loc[order], half[order]
    counts = np.bincount(w2 * 2 + hf, minlength=NWIN * 2)
    starts = np.concatenate([[0], np.cumsum(counts)])
    assert counts.reshape(-1, 2)[:, 0].max() <= T0 * 128
    assert counts.reshape(-1, 2)[:, 1].max() <= T1 * 128

    CC = CH_WIN * TW
    NC0 = CH_WIN * T0
    gsrc = np.zeros((NCHUNK, CC, 128), dtype=np.int64)
    dloc = np.full((NCHUNK, CC, 128), -1.0, dtype=np.float32)
    for wi in range(NWIN):
        ch, wl = wi // CH_WIN, wi % CH_WIN
        for h, Tn, cb in ((0, T0, wl * T0), (1, T1, NC0 + wl * T1)):
            a0, b0 = starts[wi * 2 + h], starts[wi * 2 + h + 1]
            n = b0 - a0
            sl = np.zeros(Tn * 128, dtype=np.int64)
            dl = np.full(Tn * 128, -1.0, dtype=np.float32)
            sl[:n] = ef[a0:b0]
            dl[:n] = loc2[a0:b0].astype(np.float32)
            gsrc[ch, cb:cb + Tn] = sl.reshape(Tn, 128)
            dloc[ch, cb:cb + Tn] = dl.reshape(Tn, 128)

    W0, W1 = CH_WIN * T0 * 8, CH_WIN * T1 * 8
    gidx = np.zeros((128, NCHUNK * (W0 + W1)), dtype=np.int16)
    dstloc = np.zeros((128, NCHUNK * CC), dtype=np.float32)
    for ch in range(NCHUNK):
        o = ch * (W0 + W1)
        gidx[:, o:o + W0] = _wrap_idx(gsrc[ch, :NC0].ravel())
        gidx[:, o + W0:o + W0 + W1] = _wrap_idx(gsrc[ch, NC0:].ravel())
        dstloc[:, ch * CC:(ch + 1) * CC] = dloc[ch].T
    return gidx, dstloc


def _graph_prep(cfg, src, dst):
    """Per-graph: LPT orders for all 4 shards + global j/window maps."""
    N, WIN, SHARD = cfg["N"], cfg["WIN"], cfg["SHARD"]
    deg = np.bincount(dst, minlength=N)
    jmap = np.full(N, -1, dtype=np.int64)       # node -> j within its shard
    smap = np.full(N, -1, dtype=np.int64)       # node -> shard
    ids_per_shard = []
    for s in range(4):
        ids = _stripe_ids(cfg, s)
        order = _lpt_order(cfg, deg[ids])
        ids_j = ids[order]                       # j-order node ids
        ids_per_shard.append(ids_j)
        jmap[ids_j] = np.arange(SHARD)
        smap[ids_j] = s
    return dict(jmap=jmap, smap=smap, ids=ids_per_shard)


def _core_prep(cfg, src, dst, gp, s):
    """Per-core edge structure: windows + gather positions per layer."""
    N, WIN, SHARD = cfg["N"], cfg["WIN"], cfg["SHARD"]
    ids_j = gp["ids"][s]
    jmap, smap = gp["jmap"], gp["smap"]
    # perm for x: shard nodes first (j-order), then the rest ascending
    rest = np.setdiff1d(np.arange(N), ids_j)
    perm = np.concatenate([ids_j, rest])
    inv = np.empty(N, dtype=np.int64)
    inv[perm] = np.arange(N)

    mask = smap[dst] == s
    es, ed = src[mask], dst[mask]
    j = jmap[ed]
    w = j // WIN
    loc = j % WIN
    pos1 = inv[es]                               # L1 gather row (perm order)
    pos2 = _ag_pos(cfg, smap[es], jmap[es])      # L2 row (AllGather order)
    return dict(ids=ids_j, w=w, loc=loc, pos1=pos1, pos2=pos2, perm=perm)


def _win_stats(cfg, w, pos, nwin):
    OVL, SPLIT = cfg["OVL"], cfg["SPLIT"]
    f0 = np.bincount(w[pos < OVL], minlength=nwin)
    f1 = np.bincount(w[pos >= SPLIT], minlength=nwin)
    tot = np.bincount(w, minlength=nwin)
    return f0, f1, tot


def _build_program(cfg, T):
    import concourse.bass as bass
    import concourse.bacc as bacc
    import concourse.mybir as mybir
    from concourse import tile
    from concourse.bass import exact_div

    f32, bf16, i16 = mybir.dt.float32, mybir.dt.bfloat16, mybir.dt.int16
    AF = mybir.ActivationFunctionType
    ALU = mybir.AluOpType

    N, D, HID, OUT = cfg["N"], cfg["D"], cfg["HID"], cfg["OUT"]
    WIN, CH_WIN, NCHUNK = cfg["WIN"], cfg["CH_WIN"], cfg["NCHUNK"]
    SHARD, SPLIT, STRIPE = cfg["SHARD"], cfg["SPLIT"], cfg["STRIPE"]
    NPIECE, S2CH, WINR = cfg["NPIECE"], cfg["S2CH"], cfg["WINR"]
    OVL = cfg["OVL"]
    ROWW = 256                      # bf16 elems -> 512B gather stride (L1)
    H2S = 128                       # bf16 elems -> 256B gather stride (L2)
    Z2W = OUT + 2
    CH_PER_PIECE = NCHUNK // NPIECE

    nc = bacc.Bacc("TRN2", target_bir_lowering=False, debug=False,
                   enable_asserts=True, num_devices=8, num_swdge_queues=4)

    xT_in = nc.dram_tensor("xT", [D, N], bf16, kind="ExternalInput")
    wcat1 = nc.dram_tensor("wcat1", [D, HID + 2], bf16, kind="ExternalInput")
    wcat2 = nc.dram_tensor("wcat2", [HID, Z2W], bf16, kind="ExternalInput")
    iota_in = nc.dram_tensor("iota", [128, WIN], bf16, kind="ExternalInput")
    ones_in = nc.dram_tensor("ones1", [1, 128], bf16, kind="ExternalInput")
    ident_in = nc.dram_tensor("ident", [128, 128], bf16, kind="ExternalInput")
    gidx_in = {}
    dstloc_in = {}
    for L in (1, 2):
        T0, T1 = T[L]
        wtot = NCHUNK * CH_WIN * (T0 + T1) * 8
        gidx_in[L] = nc.dram_tensor(f"gidx{L}", [128, wtot], i16,
                                    kind="ExternalInput")
        dstloc_in[L] = nc.dram_tensor(
            f"dstloc{L}", [128, NCHUNK * CH_WIN * (T0 + T1)], bf16,
            kind="ExternalInput")
    out_t = nc.dram_tensor("out", [SHARD, OUT], f32, kind="ExternalOutput")

    qctr = [0]

    def raw_gather(out_ap, in_ap, idxs_ap, num_idxs, elem_size, elem_step):
        g = nc.gpsimd
        # (i + chunk) % 4: rotate which queue-pair gets the bigger pieces
        q = (qctr[0] + qctr[0] // 4) % 4
        qctr[0] += 1
        return g.add_instruction(
            mybir.InstDMAGatherAnt(
                name=nc.get_next_instruction_name(),
                ins=[*g.lower_ap_dma(in_ap, for_custom_bir_dma=True),
                     g.lower_ap(idxs_ap),
                     g.lower_val_access(g.to_reg(num_idxs))],
                outs=[g.lower_ap(out_ap)],
                transpose=False, num_idxs=num_idxs, elem_size=elem_size,
                stride_bytes_256=exact_div(elem_step * 2, 256), gen_mode=0,
                single_packet=False, queue_num=q, sbuf_tokens_per_rank=0,
                sbuf_free_dim_per_rank=0, sbuf_free_dim_pad_per_rank=0,
                sbuf_byte_offset=0))

    def ap_of(t, dims, extra_off=0):
        a = t[:]
        return bass.AP(a.tensor, a.offset + extra_off,
                       [list(a.ap[0])] + [list(d) for d in dims])

    with tile.TileContext(nc) as tc:
        with (
            tc.tile_pool(name="const", bufs=1) as constp,
            tc.tile_pool(name="dram", bufs=1, space="DRAM") as dram,
        ):
            iota_sb = constp.tile([128, WIN], bf16, tag="iota")
            ones_sb = constp.tile([1, 128], bf16, tag="ones")
            ident_sb = constp.tile([128, 128], bf16, tag="ident")
            wc1_sb = constp.tile([D, HID + 2], bf16, tag="wc1")
            wc2_sb = constp.tile([HID, Z2W], bf16, tag="wc2")
            nc.sync.dma_start(out=iota_sb[:], in_=iota_in[:])
            nc.sync.dma_start(out=ones_sb[:], in_=ones_in[:])
            nc.sync.dma_start(out=ident_sb[:], in_=ident_in[:])
            nc.sync.dma_start(out=wc1_sb[:], in_=wcat1[:])
            nc.sync.dma_start(out=wc2_sb[:], in_=wcat2[:])
            dstloc_sb = {}
            for L in (1, 2):
                T0, T1 = T[L]
                dstloc_sb[L] = constp.tile(
                    [128, NCHUNK * CH_WIN * (T0 + T1)], bf16,
                    tag=f"dstloc{L}", name=f"dstloc_sb{L}")
                nc.sync.dma_start(out=dstloc_sb[L][:], in_=dstloc_in[L][:])

            harr = dram.tile([N, ROWW], bf16, tag="harr")
            h1p = [dram.tile([STRIPE, HID], bf16, tag=f"h1p{p}",
                             name=f"h1p{p}") for p in range(NPIECE)]
            z2cp = [dram.tile([STRIPE, H2S], bf16, tag=f"z2c{p}",
                              name=f"z2cp{p}") for p in range(NPIECE)]
            z2full = dram.tile([N, H2S], bf16, tag="z2full")
            h2arr = dram.tile([N, H2S], bf16, tag="h2arr")

            # ---------- stage 1: harr rows = [x@W1 | as | ad] ----------
            # 512-row super-chunks: 1 load DMA + 4 matmuls + 1 store DMA
            with (
                tc.tile_pool(name="s1s", bufs=3) as s1s,
                tc.tile_pool(name="s1p", bufs=2, space="PSUM") as s1p,
            ):
                SC = 512
                for c in range(-(-N // SC)):
                    r0 = c * SC
                    rn = min(SC, N - r0)
                    nt = -(-rn // 128)
                    xs = s1s.tile([128, SC], bf16, tag="xs")
                    eng_in = nc.sync if c % 2 == 0 else nc.scalar
                    nc_out = nc.scalar if c % 2 == 0 else nc.sync
                    eng_in.dma_start(out=xs[:, :rn],
                                     in_=xT_in[:, r0:r0 + rn])
                    # pad each row to ROWW elems -> harr store is contiguous
                    zs = s1s.tile([128, nt * ROWW], bf16, tag=f"zs{nt}",
                                  name=f"zs_{c}")
                    for t in range(nt):
                        tn = min(128, rn - t * 128)
                        z_ps = s1p.tile([128, HID + 2], f32, tag="zps")
                        nc.tensor.matmul(
                            out=z_ps[:tn],
                            lhsT=xs[:, t * 128:t * 128 + tn],
                            rhs=wc1_sb[:], start=True, stop=True)
                        nc.vector.tensor_copy(
                            out=zs[:tn, t * ROWW:t * ROWW + HID + 2],
                            in_=z_ps[:tn])
                    if rn % 128 == 0:
                        nc_out.dma_start(
                            out=bass.AP(harr[:].tensor,
                                        harr[:].offset + r0 * ROWW,
                                        [[ROWW, 128], [128 * ROWW, nt],
                                         [1, ROWW]]),
                            in_=zs[:].rearrange("p (t e) -> p t e", t=nt))
                    else:
                        for t in range(nt):
                            tn = min(128, rn - t * 128)
                            nc_out.dma_start(
                                out=harr[r0 + t * 128:r0 + t * 128 + tn, :],
                                in_=zs[:tn, t * ROWW:(t + 1) * ROWW])

            # ---------- stage 2 (per piece): h1 -> z2c -> AG -> h2arr --------
            def stage2_piece(p):
                with (
                    tc.tile_pool(name=f"s2s{p}", bufs=3) as s2s,
                    tc.tile_pool(name=f"s2p{p}", bufs=1, space="PSUM") as s2p,
                ):
                    for c in range(STRIPE // S2CH):
                        r0 = c * S2CH
                        hs = s2s.tile([S2CH, HID], bf16, tag="hs")
                        nc.sync.dma_start(out=hs[:],
                                          in_=h1p[p][r0:r0 + S2CH, :])
                        ht_ps = s2p.tile([128, S2CH], bf16, tag="ht")
                        nc.tensor.transpose(out=ht_ps[:, :S2CH], in_=hs[:],
                                            identity=ident_sb[:S2CH, :S2CH])
                        ht = s2s.tile([128, S2CH], bf16, tag="hts")
                        # relu(h1) folded here: relu commutes with transpose
                        nc.scalar.activation(out=ht[:], in_=ht_ps[:],
                                             func=AF.Relu)
                        z_ps = s2p.tile([S2CH, Z2W], f32, tag="z2ps")
                        nc.tensor.matmul(out=z_ps[:], lhsT=ht[:],
                                         rhs=wc2_sb[:], start=True, stop=True)
                        zs = s2s.tile([S2CH, H2S], bf16, tag="z2s")
                        nc.vector.tensor_copy(out=zs[:, :Z2W], in_=z_ps[:])
                        nc.sync.dma_start(out=z2cp[p][r0:r0 + S2CH, :],
                                          in_=zs[:])
                nc.gpsimd.collective_compute(
                    "AllGather", mybir.AluOpType.bypass,
                    replica_groups=[[0, 1, 2, 3], [4, 5, 6, 7]],
                    ins=[z2cp[p][:, :].opt()],
                    outs=[z2full[p * 4 * STRIPE:(p + 1) * 4 * STRIPE, :].opt()])
                rr0 = p * 4 * STRIPE
                nfr = 4 * STRIPE
                nc.sync.dma_start(out=h2arr[rr0:rr0 + nfr, :],
                                  in_=z2full[rr0:rr0 + nfr, :])

            # ---------- edge phase ----------
            def edge_phase(L):
                T0, T1 = T[L]
                TW = T0 + T1
                CC = CH_WIN * TW
                NC0 = CH_WIN * T0
                NC1 = CH_WIN * T1
                W0, W1 = NC0 * 8, NC1 * 8
                F = HID if L == 1 else OUT
                GE = F + 2
                src_t = harr if L == 1 else h2arr
                stride = ROWW if L == 1 else H2S
                with (
                    tc.tile_pool(name=f"ep{L}", bufs=4) as ep,
                    tc.tile_pool(name=f"rp{L}", bufs=2, space="PSUM") as rpp,
                    tc.tile_pool(name=f"ac{L}", bufs=1, space="PSUM") as accp,
                ):
                    IBW = W0 + W1
                    for ch in range(NCHUNK):
                        jbase = ch * WINR
                        piece = jbase // STRIPE
                        if ch % CH_PER_PIECE == 0:
                            # batched loads for the next CH_PER_PIECE chunks
                            ib5 = ep.tile([128, CH_PER_PIECE * IBW], i16,
                                          tag="ib5", bufs=2)
                            nc.scalar.dma_start(
                                out=ib5[:],
                                in_=gidx_in[L][:, ch * IBW:
                                               (ch + CH_PER_PIECE) * IBW])
                            adcP = ep.tile([1, STRIPE], bf16, tag="adcP",
                                           bufs=2)
                            if L == 1:
                                sap = bass.AP(harr[:].tensor,
                                              harr[:].offset + jbase * ROWW
                                              + HID + 1,
                                              [[ROWW, STRIPE], [1, 1]])
                            else:
                                zp = z2cp[piece]
                                sap = bass.AP(zp[:].tensor,
                                              zp[:].offset + OUT + 1,
                                              [[H2S, STRIPE], [1, 1]])
                            nc.scalar.dma_start(out=adcP[:], in_=sap)
                        co = (ch % CH_PER_PIECE) * IBW
                        G = ep.tile([128, CC * GE], bf16, tag="G", bufs=5)
                        G3 = G[:].rearrange("p (c e) -> p c e", e=GE)
                        # split into 4 near-equal gathers so the 4 SWDGE
                        # queue cpu-pairs stay load-balanced every chunk
                        h0a = NC0 // 2
                        for c0, c1 in ((0, h0a), (h0a, NC0)):
                            raw_gather(G3[:, c0:c1, :], src_t[:SPLIT, :GE],
                                       ib5[:, co + c0 * 8:co + c1 * 8],
                                       (c1 - c0) * 128, GE, stride)
                        h1a = NC1 - NC1 // 2
                        for c0, c1 in ((NC0, NC0 + h1a), (NC0 + h1a, NC0 + NC1)):
                            raw_gather(G3[:, c0:c1, :],
                                       src_t[OVL:OVL + SPLIT, :GE],
                                       ib5[:, co + W0 + (c0 - NC0) * 8:
                                           co + W0 + (c1 - NC0) * 8],
                                       (c1 - c0) * 128, GE, stride)
                        # emit the previous piece's stage2+AllGather here,
                        # AFTER this chunk's gathers: the collective's GpSimd
                        # engine-block then overlaps their cpu-pair execution
                        if L == 1 and ch % CH_PER_PIECE == 1 and ch > 1:
                            stage2_piece(ch // CH_PER_PIECE - 1)
                        adr_ps = rpp.tile([128, WINR], f32, tag="adr")
                        ao = (ch % CH_PER_PIECE) * WINR
                        nc.tensor.matmul(out=adr_ps[:], lhsT=ones_sb[:],
                                         rhs=adcP[:, ao:ao + WINR],
                                         start=True, stop=True)
                        adr = ep.tile([128, WINR], bf16, tag="adrs")
                        nc.scalar.copy(out=adr[:], in_=adr_ps[:])
                        # ME = alpha_src + alpha_dst  (then lrelu, exp, mask)
                        ME = ep.tile([128, CC * WIN], f32, tag="ME")
                        for Tn, cb in ((T0, 0), (T1, NC0)):
                            if Tn == 0:
                                continue
                            nc.vector.tensor_tensor(
                                out=ap_of(ME, [[Tn * WIN, CH_WIN], [WIN, Tn],
                                               [1, WIN]], cb * WIN),
                                in0=ap_of(G, [[Tn * GE, CH_WIN], [GE, Tn],
                                              [0, WIN]], cb * GE + F),
                                in1=ap_of(adr, [[WIN, CH_WIN], [0, Tn],
                                                [1, WIN]]),
                                op=ALU.add)
                        # exp(leakyrelu(e)) = max(exp(e), exp(0.2*e))
                        MXa = ep.tile([128, CC * WIN], bf16, tag="MXa")
                        nc.scalar.activation(out=MXa[:], in_=ME[:],
                                             func=AF.Exp)
                        MXb = ep.tile([128, CC * WIN], bf16, tag="MXb")
                        nc.scalar.activation(out=MXb[:], in_=ME[:],
                                             func=AF.Exp, scale=NEG_SLOPE)
                        MX = ep.tile([128, CC * WIN], bf16, tag="MX")
                        nc.vector.tensor_tensor(out=MX[:], in0=MXa[:],
                                                in1=MXb[:], op=ALU.max)
                        M0 = ep.tile([128, CC * WIN], bf16, tag="M0")
                        nc.vector.tensor_tensor(
                            out=M0[:],
                            in0=ap_of(dstloc_sb[L], [[1, CC], [0, WIN]],
                                      ch * CC),
                            in1=ap_of(iota_sb, [[0, CC], [1, WIN]]),
                            op=ALU.is_equal)
                        nc.vector.tensor_tensor(out=M0[:], in0=MX[:],
                                                in1=M0[:], op=ALU.mult)
                        nc.vector.memset(ap_of(G, [[GE, CC], [1, 1]], F), 1.0)
                        M3 = M0[:].rearrange("p (c w) -> p c w", w=WIN)
                        accA = accp.tile([WIN, 3 * GE], f32, tag="accA",
                                         bufs=2)
                        accB = accp.tile([WIN, 2 * GE], f32, tag="accB",
                                         bufs=2)
                        accs = [accA[:, 0:GE], accA[:, GE:2 * GE],
                                accA[:, 2 * GE:3 * GE],
                                accB[:, 0:GE], accB[:, GE:2 * GE]]
                        for wl in range(CH_WIN):
                            acc = accs[wl]
                            cols = ([wl * T0 + k for k in range(T0)] +
                                    [NC0 + wl * T1 + k for k in range(T1)])
                            for ci, col in enumerate(cols):
                                nc.tensor.matmul(
                                    out=acc, lhsT=M3[:, col, :],
                                    rhs=G3[:, col, :],
                                    start=(ci == 0), stop=(ci == TW - 1))
                        res5 = ep.tile([WIN, CH_WIN * F],
                                       bf16 if L == 1 else f32, tag="res5",
                                       bufs=2)
                        for wl in range(CH_WIN):
                            rcp = ep.tile([WIN, 1], f32, tag="rcp", bufs=2)
                            nc.vector.reciprocal(out=rcp[:],
                                                 in_=accs[wl][:, F:F + 1])
                            rsl = res5[:, wl * F:(wl + 1) * F]
                            nc.vector.tensor_tensor(
                                out=rsl, in0=accs[wl][:, :F],
                                in1=ap_of(rcp, [[0, F]]), op=ALU.mult)
                            # L1 relu is folded into stage2's ht copy
                        r5 = res5[:].rearrange("p (c e) -> p c e", e=F)
                        if L == 1:
                            hp = h1p[piece]
                            r0 = jbase % STRIPE
                            nc.sync.dma_start(
                                out=bass.AP(hp[:].tensor,
                                            hp[:].offset + r0 * HID,
                                            [[HID, WIN], [WIN * HID, CH_WIN],
                                             [1, HID]]),
                                in_=r5)
                        else:
                            nc.sync.dma_start(
                                out=bass.AP(out_t[:].tensor,
                                            out_t[:].offset + jbase * OUT,
                                            [[OUT, WIN], [WIN * OUT, CH_WIN],
                                             [1, OUT]]),
                                in_=r5)
                    if L == 1:
                        for p_ in range(NPIECE - 1, NPIECE):
                            stage2_piece(p_)

            edge_phase(1)
            edge_phase(2)

    nc.compile()
    return nc


_PROG_CACHE = {}


LAST_EXEC_NS = None


def _run(cfg_in, fea_mats, edge_index, W1, att_src1, att_dst1, b1,
         W2, att_src2, att_dst2, b2, trace=False):
    import ml_dtypes
    from concourse.bass_utils import run_bass_kernel_spmd

    bf = ml_dtypes.bfloat16
    cfg = _derive(cfg_in)
    N, B, OUT, SHARD = cfg["N"], cfg["B"], cfg["OUT"], cfg["SHARD"]
    NWIN = cfg["NWIN"]

    fea = np.ascontiguousarray(np.asarray(fea_mats, dtype=np.float32))
    ei = np.asarray(edge_index)
    W1 = np.asarray(W1, np.float32)
    W2 = np.asarray(W2, np.float32)
    as1 = np.asarray(att_src1, np.float32)[0]
    ad1 = np.asarray(att_dst1, np.float32)[0]
    as2 = np.asarray(att_src2, np.float32)[0]
    ad2 = np.asarray(att_dst2, np.float32)[0]
    b1 = np.asarray(b1, np.float32)
    b2 = np.asarray(b2, np.float32)
    assert not np.any(b1 != 0), "b1 != 0 unsupported in this build"

    loops = np.arange(N, dtype=np.int64)
    graphs = []
    for g in range(B):
        graphs.append((np.concatenate([ei[g, 0].astype(np.int64), loops]),
                       np.concatenate([ei[g, 1].astype(np.int64), loops])))

    gps = [_graph_prep(cfg, *graphs[g]) for g in range(B)]
    preps = [_core_prep(cfg, *graphs[c // 4], gps[c // 4], c % 4)
             for c in range(8)]
    T = {}
    for L, key in ((1, "pos1"), (2, "pos2")):
        stats = [_win_stats(cfg, pr["w"], pr[key], NWIN) for pr in preps]
        T[L] = _choose_T(cfg, stats)

    wcat1 = np.concatenate([W1, (W1 @ as1)[:, None], (W1 @ ad1)[:, None]],
                           axis=1).astype(bf)
    wcat2 = np.concatenate([W2, (W2 @ as2)[:, None], (W2 @ ad2)[:, None]],
                           axis=1).astype(bf)
    iota = np.tile(np.arange(cfg["WIN"], dtype=np.float32), (128, 1))

    in_maps = []
    for core in range(8):
        g = core // 4
        pr = preps[core]
        m = dict(xT=np.ascontiguousarray(fea[g][pr["perm"]].T).astype(bf),
                 wcat1=wcat1, wcat2=wcat2,
                 iota=iota.astype(bf),
                 ones1=np.ones((1, 128), bf),
                 ident=np.eye(128, dtype=np.float32).astype(bf))
        for L, posk in ((1, "pos1"), (2, "pos2")):
            half, eff = _assign_halves(cfg, T[L], pr["w"], pr[posk], NWIN)
            gx, dl = _layer_streams(cfg, pr["w"], pr["loc"], half, eff, *T[L])
            m[f"gidx{L}"] = gx
            m[f"dstloc{L}"] = dl.astype(bf)
        in_maps.append(m)

    key = (tuple(sorted(cfg_in.items())), T[1], T[2])
    if key not in _PROG_CACHE:
        _PROG_CACHE[key] = _build_program(cfg, T)
    nc = _PROG_CACHE[key]
    res = run_bass_kernel_spmd(nc, in_maps, list(range(8)), trace=trace)
    global LAST_EXEC_NS
    LAST_EXEC_NS = res.exec_time_ns

    out = np.zeros((B, N, OUT), dtype=np.float32)
    for core in range(8):
        g = core // 4
        out[g, preps[core]["ids"]] = res.results[core]["out"]
    if np.any(b2 != 0):
        out += b2[None, None, :]
    return out


def kernel(**inputs):
    return _run(FULL_CFG, **inputs)
